# revision 5
# baseline (speedup 1.0000x reference)
"""CountBasedReward (SimHash) Trainium2 kernel.

Computes, for full inputs
    next_observation [B, 4, 84, 84] f32, done [B, 1] f32, A [K, D] f32:
    signs  = sign(feats @ A.T)            # [B, K] SimHash codes
    counts = #identical codes per row     # [B]
    out    = done / sqrt(counts)          # [B, 1]

Strategy: data-parallel over batch across 8 NeuronCores. Each core
projects its 256-row feats shard against full A (bf16 matmul, fp32 PSUM
accumulate), maps the projection to codes in {-0.5, +0.5}, AllGathers
the 128KB of codes across cores, then counts exact code matches of its
local rows vs all 2048 rows with a second small matmul (dot == K/4 iff
codes identical), and finishes rewards = done * 1/sqrt(counts).

bf16 is safe here: code-match counting only depends on pairwise exact
equality of the 256-bit codes, which is invariant to small sign-boundary
perturbations (two distinct random rows differ in ~128 bits).
"""

import numpy as np
import ml_dtypes

import concourse.bacc as bacc
import concourse.bass as bass
import concourse.mybir as mybir
import concourse.tile as tile
from concourse.bass_utils import run_bass_kernel_spmd

BF16 = mybir.dt.bfloat16
F32 = mybir.dt.float32

N_CORES = 8
B = 2048
D = 4 * 84 * 84  # 28224
K = 256


def _d_segments(d_total, t_max=8):
    """Split the contraction dim into supertiles of t*p rows (p<=128)."""
    segs = []
    d0 = 0
    while d0 < d_total:
        left = d_total - d0
        if left >= 128:
            t = min(t_max, left // 128)
            segs.append((d0, 128, t))
            d0 += 128 * t
        else:
            segs.append((d0, left, 1))
            d0 = d_total
    return segs


def build_nc(n_cores=N_CORES, b=B, d=D, k=K, t_max=8):
    """Build + compile the SPMD Bass program (identical on all cores)."""
    bs = b // n_cores  # local batch rows per core
    assert k % 128 == 0 and bs % 128 == 0 and b % 512 == 0
    kt = k // 128      # k-bit partition tiles (2)
    mt = bs // 128     # local batch partition tiles (2)

    nc = bacc.Bacc(
        "TRN2",
        target_bir_lowering=False,
        debug=False,
        enable_asserts=False,
        num_devices=n_cores,
    )

    xt = nc.dram_tensor("xt", [d, bs], BF16, kind="ExternalInput")
    at = nc.dram_tensor("at", [d, k], BF16, kind="ExternalInput")
    dn = nc.dram_tensor("dn", [bs, 1], F32, kind="ExternalInput")
    rew = nc.dram_tensor("rew", [bs, 1], F32, kind="ExternalOutput")

    segs = _d_segments(d, t_max=t_max)
    groups = [list(range(n_cores))]

    with tile.TileContext(nc) as tc:
        with (
            tc.tile_pool(name="atp", bufs=3) as atp,
            tc.tile_pool(name="xtp", bufs=3) as xtp,
            tc.tile_pool(name="sps", bufs=1, space="PSUM") as sps,
            tc.tile_pool(name="dotp", bufs=4, space="PSUM") as dotp,
            tc.tile_pool(name="sbp", bufs=1) as sbp,
            tc.tile_pool(name="dram", bufs=1, space="DRAM") as dram,
        ):
            # ---- Phase A: S_T[j, b_local] = (A @ feats_local.T) in sign space
            s_ps = [sps.tile([128, bs], F32, name=f"s_ps{j}") for j in range(kt)]
            n_sub = sum(t for (_, _, t) in segs)
            sub = 0
            for (d0, p, t) in segs:
                att = atp.tile([128, t, k], BF16, name="att", tag="att")
                xtt = xtp.tile([128, t, bs], BF16, name="xtt", tag="xtt")
                nc.sync.dma_start(
                    out=att[:p],
                    in_=at[d0 : d0 + p * t, :].rearrange("(p t) k -> p t k", p=p, t=t),
                )
                nc.sync.dma_start(
                    out=xtt[:p],
                    in_=xt[d0 : d0 + p * t, :].rearrange("(p t) b -> p t b", p=p, t=t),
                )
                for ti in range(t):
                    sub += 1
                    for j in range(kt):
                        nc.tensor.matmul(
                            s_ps[j][:],
                            lhsT=att[:p, ti, j * 128 : (j + 1) * 128],
                            rhs=xtt[:p, ti, :],
                            start=(sub == 1),
                            stop=(sub == n_sub),
                        )

            # ---- Phase B: codes in {-0.5, +0.5}: (x > 0) - 0.5
            s_sb = []
            for j in range(kt):
                s = sbp.tile([128, bs], BF16, name=f"s_sb{j}")
                nc.vector.tensor_scalar(
                    out=s[:],
                    in0=s_ps[j][:],
                    scalar1=0.0,
                    scalar2=0.5,
                    op0=mybir.AluOpType.is_gt,
                    op1=mybir.AluOpType.subtract,
                )
                s_sb.append(s)

            # ---- Phase C: AllGather codes across cores
            cc_in = dram.tile([k, bs], BF16, name="cc_in")
            cc_out = dram.tile([n_cores * k, bs], BF16, addr_space="Shared",
                               name="cc_out")
            for j in range(kt):
                nc.sync.dma_start(out=cc_in[j * 128 : (j + 1) * 128, :], in_=s_sb[j][:])
            nc.gpsimd.collective_compute(
                "AllGather",
                mybir.AluOpType.bypass,
                replica_groups=groups,
                ins=[cc_in[:].opt()],
                outs=[cc_out[:].opt()],
            )

            # ---- Phase D: load all codes as [k_bit partitions, global batch]
            # cc_out rows are (rank r, k-bit kk) at r*k + kk; col = local b.
            # R_h partition p <-> bit h*128+p; free col j = r*bs + b = global row.
            r_sb = []
            for h in range(kt):
                r_t = sbp.tile([128, b], BF16, name=f"r_sb{h}")
                nc.sync.dma_start(
                    out=r_t.rearrange("p (r c) -> p r c", r=n_cores),
                    in_=cc_out[:].rearrange("(r kk) c -> kk r c", r=n_cores)[
                        h * 128 : (h + 1) * 128
                    ],
                )
                r_sb.append(r_t)

            # dots[b_local, b_global] = sum_k code*code; == k/4 iff identical
            thr = k / 4.0 - 0.125
            n_chunk = b // 512
            for m in range(mt):
                cmp = sbp.tile([128, b], F32, name=f"cmp{m}")
                for n in range(n_chunk):
                    dot = dotp.tile([128, 512], F32, name="dot", tag="dot")
                    for h in range(kt):
                        nc.tensor.matmul(
                            dot[:],
                            lhsT=s_sb[h][:, m * 128 : (m + 1) * 128],
                            rhs=r_sb[h][:, n * 512 : (n + 1) * 512],
                            start=(h == 0),
                            stop=(h == kt - 1),
                        )
                    nc.vector.tensor_scalar(
                        out=cmp[:, n * 512 : (n + 1) * 512],
                        in0=dot[:],
                        scalar1=thr,
                        scalar2=None,
                        op0=mybir.AluOpType.is_ge,
                    )
                cnt = sbp.tile([128, 1], F32, name=f"cnt{m}")
                nc.vector.tensor_reduce(
                    out=cnt[:], in_=cmp[:], axis=mybir.AxisListType.X,
                    op=mybir.AluOpType.add,
                )
                # rewards = done / sqrt(counts)
                sq = sbp.tile([128, 1], F32, name=f"sq{m}")
                nc.scalar.activation(sq[:], cnt[:], mybir.ActivationFunctionType.Sqrt)
                inv = sbp.tile([128, 1], F32, name=f"inv{m}")
                nc.vector.reciprocal(inv[:], sq[:])
                dna = sbp.tile([128, 1], F32, name=f"dna{m}")
                nc.sync.dma_start(out=dna[:], in_=dn[m * 128 : (m + 1) * 128, :])
                rw = sbp.tile([128, 1], F32, name=f"rw{m}")
                nc.vector.tensor_mul(rw[:], inv[:], dna[:])
                nc.sync.dma_start(out=rew[m * 128 : (m + 1) * 128, :], in_=rw[:])

    nc.compile()
    return nc


_CACHE = {}


def _get_nc():
    if "nc" not in _CACHE:
        _CACHE["nc"] = build_nc()
    return _CACHE["nc"]


def kernel(next_observation, done, A):
    """Full inputs in, full output out. Shards across 8 cores internally."""
    nc = _get_nc()
    bs = B // N_CORES

    feats = np.asarray(next_observation).reshape(B, D)
    at_h = np.ascontiguousarray(np.asarray(A).T).astype(ml_dtypes.bfloat16)
    done_h = np.asarray(done).astype(np.float32)

    in_maps = []
    for c in range(N_CORES):
        xt_c = np.ascontiguousarray(feats[c * bs : (c + 1) * bs, :].T).astype(
            ml_dtypes.bfloat16
        )
        in_maps.append({"xt": xt_c, "at": at_h, "dn": done_h[c * bs : (c + 1) * bs]})

    res = run_bass_kernel_spmd(
        nc, in_maps, core_ids=list(range(N_CORES)), **_CACHE.get("run_kwargs", {})
    )
    _CACHE["last_results"] = res
    out = np.concatenate(
        [res.results[c]["rew"] for c in range(N_CORES)], axis=0
    ).astype(np.float32)
    return out


def benchmark(next_observation, done, A, n_iters=20):
    """Time repeated NEFF executions with device-resident inputs.

    Returns (min, median) wall seconds per execution (dispatch + HW exec),
    measured on the jitted shard_map executable built once.
    """
    import time
    import jax
    from jax.sharding import Mesh, PartitionSpec, NamedSharding
    from jax.experimental.shard_map import shard_map
    from concourse import bass2jax

    nc = _get_nc()
    bs = B // N_CORES
    feats = np.asarray(next_observation).reshape(B, D)
    at_h = np.ascontiguousarray(np.asarray(A).T).astype(ml_dtypes.bfloat16)
    done_h = np.asarray(done).astype(np.float32)
    in_maps = []
    for c in range(N_CORES):
        xt_c = np.ascontiguousarray(feats[c * bs : (c + 1) * bs, :].T).astype(
            ml_dtypes.bfloat16
        )
        in_maps.append({"xt": xt_c, "at": at_h, "dn": done_h[c * bs : (c + 1) * bs]})

    bass2jax.install_neuronx_cc_hook()
    import concourse.mybir as mb

    in_names, out_names, out_avals, zero_outs = [], [], [], []
    partition_name = nc.partition_id_tensor.name if nc.partition_id_tensor else None
    for alloc in nc.m.functions[0].allocations:
        if not isinstance(alloc, mb.MemoryLocationSet):
            continue
        name = alloc.memorylocations[0].name
        if alloc.kind == "ExternalInput":
            if name != partition_name:
                in_names.append(name)
        elif alloc.kind == "ExternalOutput":
            out_names.append(name)
            shape = tuple(alloc.tensor_shape)
            dtype = mb.dt.np(alloc.dtype)
            out_avals.append(jax.core.ShapedArray(shape, dtype))
            zero_outs.append(np.zeros(shape, dtype))
    n_params = len(in_names)
    all_in_names = list(in_names) + list(out_names)
    if partition_name is not None:
        all_in_names.append(partition_name)

    def _body(*args):
        operands = list(args)
        if partition_name is not None:
            operands.append(bass2jax.partition_id_tensor())
        return tuple(
            bass2jax._bass_exec_p.bind(
                *operands,
                out_avals=tuple(out_avals),
                in_names=tuple(all_in_names),
                out_names=tuple(out_names),
                lowering_input_output_aliases=(),
                sim_require_finite=True,
                sim_require_nnan=True,
                nc=nc,
            )
        )

    devices = jax.devices()[:N_CORES]
    mesh = Mesh(np.asarray(devices), ("core",))
    n_outs = len(out_names)
    sharded = jax.jit(
        shard_map(
            _body,
            mesh=mesh,
            in_specs=(PartitionSpec("core"),) * (n_params + n_outs),
            out_specs=(PartitionSpec("core"),) * n_outs,
            check_rep=False,
        ),
        donate_argnums=tuple(range(n_params, n_params + n_outs)),
        keep_unused=True,
    )
    sh = NamedSharding(mesh, PartitionSpec("core"))
    dev_in = [
        jax.device_put(
            np.concatenate([np.asarray(m[name]) for m in in_maps], axis=0), sh
        )
        for name in in_names
    ]

    def zeros():
        return [
            jax.device_put(
                np.zeros((N_CORES * z.shape[0], *z.shape[1:]), z.dtype), sh
            )
            for z in zero_outs
        ]

    # warmup (compiles)
    out = sharded(*dev_in, *zeros())
    jax.block_until_ready(out)

    times = []
    for _ in range(n_iters):
        zs = zeros()
        jax.block_until_ready(zs)
        t0 = time.perf_counter()
        out = sharded(*dev_in, *zs)
        jax.block_until_ready(out)
        times.append(time.perf_counter() - t0)
    times.sort()
    return times[0], times[len(times) // 2]


# revision 17
# speedup vs baseline: 622.0738x; 622.0738x over previous
"""CountBasedReward (SimHash) Trainium2 kernel.

Computes, for full inputs
    next_observation [B, 4, 84, 84] f32, done [B, 1] f32, A [K, D] f32:
    signs  = sign(feats @ A.T)            # [B, K] SimHash codes
    counts = #identical codes per row     # [B]
    out    = done / sqrt(counts)          # [B, 1]

Strategy: data-parallel over batch across 8 NeuronCores. Each core
projects its 256-row feats shard against full A (fp8 DoubleRow matmuls,
fp32 PSUM accumulate), maps the projection to codes in {-0.5, +0.5},
AllGathers the codes across cores (fp8, 64KB/rank; a tiny warmup
collective at kernel start absorbs the ncfw control-plane setup), then
counts exact code matches of its local rows vs all 2048 rows with a
second small matmul (dot == K/4 iff codes identical), and finishes
rewards = done * 1/sqrt(counts).

fp8 is safe here: code-match counting only depends on pairwise exact
equality of the 256-bit codes, which is invariant to small sign-boundary
perturbations (two distinct random rows differ in ~128 bits), so the
final rewards match the fp32 reference bit-robustly.
"""

import numpy as np

import concourse.bacc as bacc
import concourse.bass as bass
import concourse.mybir as mybir
import concourse.tile as tile
from concourse.bass_utils import run_bass_kernel_spmd

BF16 = mybir.dt.bfloat16
F32 = mybir.dt.float32
F8 = mybir.dt.float8e4
IN_DT = F8  # projection operand dtype (codes are equality-robust to fp8)

N_CORES = 8
B = 2048
D = 4 * 84 * 84  # 28224
K = 256


def _d_segments(d_total, t_max=8):
    """Split the contraction dim into supertiles of t*p rows (p<=128).

    Ramp the first supertiles small so the first matmuls start as soon
    as the first small DMAs land, instead of waiting for a 512KB pair.
    """
    segs = []
    d0 = 0
    ramp = [2, 2, 4]  # even t so supertiles split into DoubleRow pairs
    while d0 < d_total:
        left = d_total - d0
        if left >= 256:
            t = min(ramp.pop(0) if ramp else t_max, (left // 256) * 2)
            segs.append((d0, 128, t))
            d0 += 128 * t
        elif left >= 128:
            segs.append((d0, 128, 1))
            d0 += 128
        else:
            segs.append((d0, left, 1))
            d0 = d_total
    return segs


def build_nc(n_cores=N_CORES, b=B, d=D, k=K, t_max=16):
    """Build + compile the SPMD Bass program (identical on all cores)."""
    bs = b // n_cores  # local batch rows per core
    assert k % 128 == 0 and bs % 128 == 0 and b % 512 == 0
    kt = k // 128      # k-bit partition tiles (2)
    mt = bs // 128     # local batch partition tiles (2)

    nc = bacc.Bacc(
        "TRN2",
        target_bir_lowering=False,
        debug=False,
        enable_asserts=False,
        num_devices=n_cores,
    )

    xt = nc.dram_tensor("xt", [d, bs], IN_DT, kind="ExternalInput")
    at = nc.dram_tensor("at", [d, k], IN_DT, kind="ExternalInput")
    dn = nc.dram_tensor("dn", [bs, 1], F32, kind="ExternalInput")
    rew = nc.dram_tensor("rew", [bs, 1], F32, kind="ExternalOutput")

    segs = _d_segments(d, t_max=t_max)
    groups = [list(range(n_cores))]

    with tile.TileContext(nc) as tc:
        with (
            tc.tile_pool(name="atp", bufs=6) as atp,
            tc.tile_pool(name="xtp", bufs=6) as xtp,
            tc.tile_pool(name="sps", bufs=1, space="PSUM") as sps,
            tc.tile_pool(name="dotp", bufs=4, space="PSUM") as dotp,
            tc.tile_pool(name="sbp", bufs=1) as sbp,
            tc.tile_pool(name="dram", bufs=1, space="DRAM") as dram,
        ):
            # Warmup collective: absorbs the ncfw control-plane setup (~15us
            # observed trigger->mesh-begin latency) concurrently with phase A
            # so the real AllGather below starts hot.
            warm_in = dram.tile([16, 1], F32, name="warm_in")
            warm_out = dram.tile([n_cores * 16, 1], F32, addr_space="Shared",
                                 name="warm_out")
            nc.sync.dma_start(out=warm_in[:], in_=dn[0:16, :])
            nc.gpsimd.collective_compute(
                "AllGather",
                mybir.AluOpType.bypass,
                replica_groups=groups,
                ins=[warm_in[:].opt()],
                outs=[warm_out[:].opt()],
            )

            # ---- Phase A: S_T[j, b_local] = (A @ feats_local.T) in sign space
            # Full-p supertiles have even t and run DoubleRow fp8 matmuls
            # (2 contraction rows per PE cell); odd tails run normal mode.
            s_ps = [sps.tile([128, bs], F32, name=f"s_ps{j}") for j in range(kt)]
            n_mm = sum((t // 2 if (p == 128 and t % 2 == 0) else t)
                       for (_, p, t) in segs)
            mm = 0
            for (d0, p, t) in segs:
                att = atp.tile([128, t, k], IN_DT, name="att", tag="att")
                xtt = xtp.tile([128, t, bs], IN_DT, name="xtt", tag="xtt")
                nc.sync.dma_start(
                    out=att[:p],
                    in_=at[d0 : d0 + p * t, :].rearrange("(p t) k -> p t k", p=p, t=t),
                )
                nc.sync.dma_start(
                    out=xtt[:p],
                    in_=xt[d0 : d0 + p * t, :].rearrange("(p t) b -> p t b", p=p, t=t),
                )
                if p == 128 and t % 2 == 0:
                    for tp in range(t // 2):
                        mm += 1
                        for j in range(kt):
                            nc.tensor.matmul(
                                s_ps[j][:],
                                lhsT=att[:, 2 * tp : 2 * tp + 2,
                                         j * 128 : (j + 1) * 128],
                                rhs=xtt[:, 2 * tp : 2 * tp + 2, :],
                                start=(mm == 1),
                                stop=(mm == n_mm),
                                perf_mode=mybir.MatmulPerfMode.DoubleRow,
                            )
                else:
                    for ti in range(t):
                        mm += 1
                        for j in range(kt):
                            nc.tensor.matmul(
                                s_ps[j][:],
                                lhsT=att[:p, ti, j * 128 : (j + 1) * 128],
                                rhs=xtt[:p, ti, :],
                                start=(mm == 1),
                                stop=(mm == n_mm),
                            )

            # ---- Phase B: codes in {-0.5, +0.5}: (x > 0) - 0.5
            s_sb = []
            for j in range(kt):
                s = sbp.tile([128, bs], F8, name=f"s_sb{j}")
                nc.vector.tensor_scalar(
                    out=s[:],
                    in0=s_ps[j][:],
                    scalar1=0.0,
                    scalar2=0.5,
                    op0=mybir.AluOpType.is_gt,
                    op1=mybir.AluOpType.subtract,
                )
                s_sb.append(s)

            # ---- Phase C: AllGather codes across cores
            cc_in = dram.tile([k, bs], F8, name="cc_in")
            cc_out = dram.tile([n_cores * k, bs], F8, addr_space="Shared",
                               name="cc_out")
            for j in range(kt):
                nc.gpsimd.dma_start(out=cc_in[j * 128 : (j + 1) * 128, :], in_=s_sb[j][:])
            nc.gpsimd.collective_compute(
                "AllGather",
                mybir.AluOpType.bypass,
                replica_groups=groups,
                ins=[cc_in[:].opt()],
                outs=[cc_out[:].opt()],
            )

            # ---- Phase D: load all codes as [k_bit partitions, global batch]
            # cc_out rows are (rank r, k-bit kk) at r*k + kk; col = local b.
            # R_h partition p <-> bit h*128+p; free col j = r*bs + b = global row.
            r_sb = []
            for h in range(kt):
                r_t = sbp.tile([128, b], F8, name=f"r_sb{h}")
                nc.gpsimd.dma_start(
                    out=r_t.rearrange("p (r c) -> p r c", r=n_cores),
                    in_=cc_out[:].rearrange("(r kk) c -> kk r c", r=n_cores)[
                        h * 128 : (h + 1) * 128
                    ],
                )
                r_sb.append(r_t)

            # dots[b_local, b_global] = sum_k code*code; == k/4 iff identical
            thr = k / 4.0 - 0.125
            n_chunk = b // 512
            for m in range(mt):
                acc = sbp.tile([128, n_chunk], F32, name=f"acc{m}")
                for n in range(n_chunk):
                    dot = dotp.tile([128, 512], F32, name="dot", tag="dot")
                    for h in range(kt):
                        nc.tensor.matmul(
                            dot[:],
                            lhsT=s_sb[h][:, m * 128 : (m + 1) * 128],
                            rhs=r_sb[h][:, n * 512 : (n + 1) * 512],
                            start=(h == 0),
                            stop=(h == kt - 1),
                        )
                    # fused: cmp = (dot >= thr) and acc[:, n] = sum(cmp)
                    cmp = sbp.tile([128, 512], BF16, name="cmp", tag="cmp", bufs=2)
                    nc.vector.tensor_scalar(
                        out=cmp[:],
                        in0=dot[:],
                        scalar1=thr,
                        scalar2=None,
                        op0=mybir.AluOpType.is_ge,
                        op1=mybir.AluOpType.add,
                        accum_out=acc[:, n : n + 1],
                    )
                cnt = sbp.tile([128, 1], F32, name=f"cnt{m}")
                nc.vector.tensor_reduce(
                    out=cnt[:], in_=acc[:], axis=mybir.AxisListType.X,
                    op=mybir.AluOpType.add,
                )
                # rewards = done / sqrt(counts)
                sq = sbp.tile([128, 1], F32, name=f"sq{m}")
                nc.scalar.activation(sq[:], cnt[:], mybir.ActivationFunctionType.Sqrt)
                inv = sbp.tile([128, 1], F32, name=f"inv{m}")
                nc.vector.reciprocal(inv[:], sq[:])
                dna = sbp.tile([128, 1], F32, name=f"dna{m}")
                nc.sync.dma_start(out=dna[:], in_=dn[m * 128 : (m + 1) * 128, :])
                rw = sbp.tile([128, 1], F32, name=f"rw{m}")
                nc.vector.tensor_mul(rw[:], inv[:], dna[:])
                nc.sync.dma_start(out=rew[m * 128 : (m + 1) * 128, :], in_=rw[:])

    nc.compile()
    return nc


_CACHE = {}


def _get_nc():
    if "nc" not in _CACHE:
        _CACHE["nc"] = build_nc()
    return _CACHE["nc"]


def kernel(next_observation, done, A):
    """Full inputs in, full output out. Shards across 8 cores internally."""
    nc = _get_nc()
    bs = B // N_CORES

    np_in = mybir.dt.np(IN_DT)
    feats = np.asarray(next_observation).reshape(B, D)
    at_h = np.ascontiguousarray(np.asarray(A).T).astype(np_in)
    done_h = np.asarray(done).astype(np.float32)

    in_maps = []
    for c in range(N_CORES):
        xt_c = np.ascontiguousarray(feats[c * bs : (c + 1) * bs, :].T).astype(np_in)
        in_maps.append({"xt": xt_c, "at": at_h, "dn": done_h[c * bs : (c + 1) * bs]})

    res = run_bass_kernel_spmd(
        nc, in_maps, core_ids=list(range(N_CORES)), **_CACHE.get("run_kwargs", {})
    )
    _CACHE["last_results"] = res
    out = np.concatenate(
        [res.results[c]["rew"] for c in range(N_CORES)], axis=0
    ).astype(np.float32)
    return out


def benchmark(next_observation, done, A, n_iters=20):
    """Time repeated NEFF executions with device-resident inputs.

    Returns (min, median) wall seconds per execution (dispatch + HW exec),
    measured on the jitted shard_map executable built once.
    """
    import time
    import jax
    from jax.sharding import Mesh, PartitionSpec, NamedSharding
    from jax.experimental.shard_map import shard_map
    from concourse import bass2jax

    nc = _get_nc()
    bs = B // N_CORES
    np_in = mybir.dt.np(IN_DT)
    feats = np.asarray(next_observation).reshape(B, D)
    at_h = np.ascontiguousarray(np.asarray(A).T).astype(np_in)
    done_h = np.asarray(done).astype(np.float32)
    in_maps = []
    for c in range(N_CORES):
        xt_c = np.ascontiguousarray(feats[c * bs : (c + 1) * bs, :].T).astype(np_in)
        in_maps.append({"xt": xt_c, "at": at_h, "dn": done_h[c * bs : (c + 1) * bs]})

    bass2jax.install_neuronx_cc_hook()
    import concourse.mybir as mb

    in_names, out_names, out_avals, zero_outs = [], [], [], []
    partition_name = nc.partition_id_tensor.name if nc.partition_id_tensor else None
    for alloc in nc.m.functions[0].allocations:
        if not isinstance(alloc, mb.MemoryLocationSet):
            continue
        name = alloc.memorylocations[0].name
        if alloc.kind == "ExternalInput":
            if name != partition_name:
                in_names.append(name)
        elif alloc.kind == "ExternalOutput":
            out_names.append(name)
            shape = tuple(alloc.tensor_shape)
            dtype = mb.dt.np(alloc.dtype)
            out_avals.append(jax.core.ShapedArray(shape, dtype))
            zero_outs.append(np.zeros(shape, dtype))
    n_params = len(in_names)
    all_in_names = list(in_names) + list(out_names)
    if partition_name is not None:
        all_in_names.append(partition_name)

    def _body(*args):
        operands = list(args)
        if partition_name is not None:
            operands.append(bass2jax.partition_id_tensor())
        return tuple(
            bass2jax._bass_exec_p.bind(
                *operands,
                out_avals=tuple(out_avals),
                in_names=tuple(all_in_names),
                out_names=tuple(out_names),
                lowering_input_output_aliases=(),
                sim_require_finite=True,
                sim_require_nnan=True,
                nc=nc,
            )
        )

    devices = jax.devices()[:N_CORES]
    mesh = Mesh(np.asarray(devices), ("core",))
    n_outs = len(out_names)
    sharded = jax.jit(
        shard_map(
            _body,
            mesh=mesh,
            in_specs=(PartitionSpec("core"),) * (n_params + n_outs),
            out_specs=(PartitionSpec("core"),) * n_outs,
            check_rep=False,
        ),
        donate_argnums=tuple(range(n_params, n_params + n_outs)),
        keep_unused=True,
    )
    sh = NamedSharding(mesh, PartitionSpec("core"))
    dev_in = [
        jax.device_put(
            np.concatenate([np.asarray(m[name]) for m in in_maps], axis=0), sh
        )
        for name in in_names
    ]

    def zeros():
        return [
            jax.device_put(
                np.zeros((N_CORES * z.shape[0], *z.shape[1:]), z.dtype), sh
            )
            for z in zero_outs
        ]

    # warmup (compiles)
    out = sharded(*dev_in, *zeros())
    jax.block_until_ready(out)

    times = []
    for _ in range(n_iters):
        zs = zeros()
        jax.block_until_ready(zs)
        t0 = time.perf_counter()
        out = sharded(*dev_in, *zs)
        jax.block_until_ready(out)
        times.append(time.perf_counter() - t0)
    times.sort()
    return times[0], times[len(times) // 2]


# revision 18
# speedup vs baseline: 629.0628x; 1.0112x over previous
"""CountBasedReward (SimHash) Trainium2 kernel.

Computes, for full inputs
    next_observation [B, 4, 84, 84] f32, done [B, 1] f32, A [K, D] f32:
    signs  = sign(feats @ A.T)            # [B, K] SimHash codes
    counts = #identical codes per row     # [B]
    out    = done / sqrt(counts)          # [B, 1]

Strategy: data-parallel over batch across 8 NeuronCores. Each core
projects its 256-row feats shard against full A (fp8 DoubleRow matmuls,
fp32 PSUM accumulate), maps the projection to codes in {-0.5, +0.5},
AllGathers the codes across cores (fp8, 64KB/rank; a tiny warmup
collective at kernel start absorbs the ncfw control-plane setup), then
counts exact code matches of its local rows vs all 2048 rows with a
second small matmul (dot == K/4 iff codes identical), and finishes
rewards = done * 1/sqrt(counts).

fp8 is safe here: code-match counting only depends on pairwise exact
equality of the 256-bit codes, which is invariant to small sign-boundary
perturbations (two distinct random rows differ in ~128 bits), so the
final rewards match the fp32 reference bit-robustly.
"""

import numpy as np

import concourse.bacc as bacc
import concourse.bass as bass
import concourse.mybir as mybir
import concourse.tile as tile
from concourse.bass_utils import run_bass_kernel_spmd

BF16 = mybir.dt.bfloat16
F32 = mybir.dt.float32
F8 = mybir.dt.float8e4
IN_DT = F8  # projection operand dtype (codes are equality-robust to fp8)

N_CORES = 8
B = 2048
D = 4 * 84 * 84  # 28224
K = 256


def _d_segments(d_total, t_max=8):
    """Split the contraction dim into supertiles of t*p rows (p<=128).

    Ramp the first supertiles small so the first matmuls start as soon
    as the first small DMAs land, instead of waiting for a 512KB pair.
    """
    segs = []
    d0 = 0
    ramp = [2, 2, 4]  # even t so supertiles split into DoubleRow pairs
    while d0 < d_total:
        left = d_total - d0
        if left >= 256:
            t = min(ramp.pop(0) if ramp else t_max, (left // 256) * 2)
            segs.append((d0, 128, t))
            d0 += 128 * t
        elif left >= 128:
            segs.append((d0, 128, 1))
            d0 += 128
        else:
            segs.append((d0, left, 1))
            d0 = d_total
    return segs


def build_nc(n_cores=N_CORES, b=B, d=D, k=K, t_max=16):
    """Build + compile the SPMD Bass program (identical on all cores)."""
    bs = b // n_cores  # local batch rows per core
    assert k % 128 == 0 and bs % 128 == 0 and b % 512 == 0
    kt = k // 128      # k-bit partition tiles (2)
    mt = bs // 128     # local batch partition tiles (2)

    nc = bacc.Bacc(
        "TRN2",
        target_bir_lowering=False,
        debug=False,
        enable_asserts=False,
        num_devices=n_cores,
    )

    xt = nc.dram_tensor("xt", [d, bs], IN_DT, kind="ExternalInput")
    at = nc.dram_tensor("at", [d, k], IN_DT, kind="ExternalInput")
    dn = nc.dram_tensor("dn", [bs, 1], F32, kind="ExternalInput")
    rew = nc.dram_tensor("rew", [bs, 1], F32, kind="ExternalOutput")

    segs = _d_segments(d, t_max=t_max)
    groups = [list(range(n_cores))]

    with tile.TileContext(nc) as tc:
        with (
            tc.tile_pool(name="atp", bufs=6) as atp,
            tc.tile_pool(name="xtp", bufs=6) as xtp,
            tc.tile_pool(name="sps", bufs=1, space="PSUM") as sps,
            tc.tile_pool(name="dotp", bufs=4, space="PSUM") as dotp,
            tc.tile_pool(name="sbp", bufs=1) as sbp,
            tc.tile_pool(name="dram", bufs=1, space="DRAM") as dram,
        ):
            # Warmup collective: absorbs the ncfw control-plane setup (~15us
            # observed trigger->mesh-begin latency) concurrently with phase A
            # so the real AllGather below starts hot.
            warm_in = dram.tile([16, 1], F32, name="warm_in")
            warm_out = dram.tile([n_cores * 16, 1], F32, addr_space="Shared",
                                 name="warm_out")
            nc.sync.dma_start(out=warm_in[:], in_=dn[0:16, :])
            nc.gpsimd.collective_compute(
                "AllGather",
                mybir.AluOpType.bypass,
                replica_groups=groups,
                ins=[warm_in[:].opt()],
                outs=[warm_out[:].opt()],
            )

            # ---- Phase A: S_T[j, b_local] = (A @ feats_local.T) in sign space
            # Full-p supertiles have even t and run DoubleRow fp8 matmuls
            # (2 contraction rows per PE cell); odd tails run normal mode.
            s_ps = [sps.tile([128, bs], F32, name=f"s_ps{j}") for j in range(kt)]
            n_mm = sum((t // 2 if (p == 128 and t % 2 == 0) else t)
                       for (_, p, t) in segs)
            mm = 0
            for (d0, p, t) in segs:
                att = atp.tile([128, t, k], IN_DT, name="att", tag="att")
                xtt = xtp.tile([128, t, bs], IN_DT, name="xtt", tag="xtt")
                nc.scalar.dma_start(
                    out=att[:p],
                    in_=at[d0 : d0 + p * t, :].rearrange("(p t) k -> p t k", p=p, t=t),
                )
                nc.sync.dma_start(
                    out=xtt[:p],
                    in_=xt[d0 : d0 + p * t, :].rearrange("(p t) b -> p t b", p=p, t=t),
                )
                if p == 128 and t % 2 == 0:
                    for tp in range(t // 2):
                        mm += 1
                        for j in range(kt):
                            nc.tensor.matmul(
                                s_ps[j][:],
                                lhsT=att[:, 2 * tp : 2 * tp + 2,
                                         j * 128 : (j + 1) * 128],
                                rhs=xtt[:, 2 * tp : 2 * tp + 2, :],
                                start=(mm == 1),
                                stop=(mm == n_mm),
                                perf_mode=mybir.MatmulPerfMode.DoubleRow,
                            )
                else:
                    for ti in range(t):
                        mm += 1
                        for j in range(kt):
                            nc.tensor.matmul(
                                s_ps[j][:],
                                lhsT=att[:p, ti, j * 128 : (j + 1) * 128],
                                rhs=xtt[:p, ti, :],
                                start=(mm == 1),
                                stop=(mm == n_mm),
                            )

            # ---- Phase B: codes in {-0.5, +0.5}: (x > 0) - 0.5
            s_sb = []
            for j in range(kt):
                s = sbp.tile([128, bs], F8, name=f"s_sb{j}")
                nc.vector.tensor_scalar(
                    out=s[:],
                    in0=s_ps[j][:],
                    scalar1=0.0,
                    scalar2=0.5,
                    op0=mybir.AluOpType.is_gt,
                    op1=mybir.AluOpType.subtract,
                )
                s_sb.append(s)

            # ---- Phase C: AllGather codes across cores
            cc_in = dram.tile([k, bs], F8, name="cc_in")
            cc_out = dram.tile([n_cores * k, bs], F8, addr_space="Shared",
                               name="cc_out")
            for j in range(kt):
                nc.gpsimd.dma_start(out=cc_in[j * 128 : (j + 1) * 128, :], in_=s_sb[j][:])
            nc.gpsimd.collective_compute(
                "AllGather",
                mybir.AluOpType.bypass,
                replica_groups=groups,
                ins=[cc_in[:].opt()],
                outs=[cc_out[:].opt()],
            )

            # ---- Phase D: load all codes as [k_bit partitions, global batch]
            # cc_out rows are (rank r, k-bit kk) at r*k + kk; col = local b.
            # R_h partition p <-> bit h*128+p; free col j = r*bs + b = global row.
            r_sb = []
            for h in range(kt):
                r_t = sbp.tile([128, b], F8, name=f"r_sb{h}")
                nc.gpsimd.dma_start(
                    out=r_t.rearrange("p (r c) -> p r c", r=n_cores),
                    in_=cc_out[:].rearrange("(r kk) c -> kk r c", r=n_cores)[
                        h * 128 : (h + 1) * 128
                    ],
                )
                r_sb.append(r_t)

            # dots[b_local, b_global] = sum_k code*code; == k/4 iff identical
            thr = k / 4.0 - 0.125
            n_chunk = b // 512
            for m in range(mt):
                acc = sbp.tile([128, n_chunk], F32, name=f"acc{m}")
                for n in range(n_chunk):
                    dot = dotp.tile([128, 512], F32, name="dot", tag="dot")
                    for h in range(kt):
                        nc.tensor.matmul(
                            dot[:],
                            lhsT=s_sb[h][:, m * 128 : (m + 1) * 128],
                            rhs=r_sb[h][:, n * 512 : (n + 1) * 512],
                            start=(h == 0),
                            stop=(h == kt - 1),
                        )
                    # fused: cmp = (dot >= thr) and acc[:, n] = sum(cmp)
                    cmp = sbp.tile([128, 512], BF16, name="cmp", tag="cmp", bufs=2)
                    nc.vector.tensor_scalar(
                        out=cmp[:],
                        in0=dot[:],
                        scalar1=thr,
                        scalar2=None,
                        op0=mybir.AluOpType.is_ge,
                        op1=mybir.AluOpType.add,
                        accum_out=acc[:, n : n + 1],
                    )
                cnt = sbp.tile([128, 1], F32, name=f"cnt{m}")
                nc.vector.tensor_reduce(
                    out=cnt[:], in_=acc[:], axis=mybir.AxisListType.X,
                    op=mybir.AluOpType.add,
                )
                # rewards = done / sqrt(counts)
                sq = sbp.tile([128, 1], F32, name=f"sq{m}")
                nc.scalar.activation(sq[:], cnt[:], mybir.ActivationFunctionType.Sqrt)
                inv = sbp.tile([128, 1], F32, name=f"inv{m}")
                nc.vector.reciprocal(inv[:], sq[:])
                dna = sbp.tile([128, 1], F32, name=f"dna{m}")
                nc.sync.dma_start(out=dna[:], in_=dn[m * 128 : (m + 1) * 128, :])
                rw = sbp.tile([128, 1], F32, name=f"rw{m}")
                nc.vector.tensor_mul(rw[:], inv[:], dna[:])
                nc.sync.dma_start(out=rew[m * 128 : (m + 1) * 128, :], in_=rw[:])

    nc.compile()
    return nc


_CACHE = {}


def _get_nc():
    if "nc" not in _CACHE:
        _CACHE["nc"] = build_nc()
    return _CACHE["nc"]


def kernel(next_observation, done, A):
    """Full inputs in, full output out. Shards across 8 cores internally."""
    nc = _get_nc()
    bs = B // N_CORES

    np_in = mybir.dt.np(IN_DT)
    feats = np.asarray(next_observation).reshape(B, D)
    at_h = np.ascontiguousarray(np.asarray(A).T).astype(np_in)
    done_h = np.asarray(done).astype(np.float32)

    in_maps = []
    for c in range(N_CORES):
        xt_c = np.ascontiguousarray(feats[c * bs : (c + 1) * bs, :].T).astype(np_in)
        in_maps.append({"xt": xt_c, "at": at_h, "dn": done_h[c * bs : (c + 1) * bs]})

    res = run_bass_kernel_spmd(
        nc, in_maps, core_ids=list(range(N_CORES)), **_CACHE.get("run_kwargs", {})
    )
    _CACHE["last_results"] = res
    out = np.concatenate(
        [res.results[c]["rew"] for c in range(N_CORES)], axis=0
    ).astype(np.float32)
    return out


def benchmark(next_observation, done, A, n_iters=20):
    """Time repeated NEFF executions with device-resident inputs.

    Returns (min, median) wall seconds per execution (dispatch + HW exec),
    measured on the jitted shard_map executable built once.
    """
    import time
    import jax
    from jax.sharding import Mesh, PartitionSpec, NamedSharding
    from jax.experimental.shard_map import shard_map
    from concourse import bass2jax

    nc = _get_nc()
    bs = B // N_CORES
    np_in = mybir.dt.np(IN_DT)
    feats = np.asarray(next_observation).reshape(B, D)
    at_h = np.ascontiguousarray(np.asarray(A).T).astype(np_in)
    done_h = np.asarray(done).astype(np.float32)
    in_maps = []
    for c in range(N_CORES):
        xt_c = np.ascontiguousarray(feats[c * bs : (c + 1) * bs, :].T).astype(np_in)
        in_maps.append({"xt": xt_c, "at": at_h, "dn": done_h[c * bs : (c + 1) * bs]})

    bass2jax.install_neuronx_cc_hook()
    import concourse.mybir as mb

    in_names, out_names, out_avals, zero_outs = [], [], [], []
    partition_name = nc.partition_id_tensor.name if nc.partition_id_tensor else None
    for alloc in nc.m.functions[0].allocations:
        if not isinstance(alloc, mb.MemoryLocationSet):
            continue
        name = alloc.memorylocations[0].name
        if alloc.kind == "ExternalInput":
            if name != partition_name:
                in_names.append(name)
        elif alloc.kind == "ExternalOutput":
            out_names.append(name)
            shape = tuple(alloc.tensor_shape)
            dtype = mb.dt.np(alloc.dtype)
            out_avals.append(jax.core.ShapedArray(shape, dtype))
            zero_outs.append(np.zeros(shape, dtype))
    n_params = len(in_names)
    all_in_names = list(in_names) + list(out_names)
    if partition_name is not None:
        all_in_names.append(partition_name)

    def _body(*args):
        operands = list(args)
        if partition_name is not None:
            operands.append(bass2jax.partition_id_tensor())
        return tuple(
            bass2jax._bass_exec_p.bind(
                *operands,
                out_avals=tuple(out_avals),
                in_names=tuple(all_in_names),
                out_names=tuple(out_names),
                lowering_input_output_aliases=(),
                sim_require_finite=True,
                sim_require_nnan=True,
                nc=nc,
            )
        )

    devices = jax.devices()[:N_CORES]
    mesh = Mesh(np.asarray(devices), ("core",))
    n_outs = len(out_names)
    sharded = jax.jit(
        shard_map(
            _body,
            mesh=mesh,
            in_specs=(PartitionSpec("core"),) * (n_params + n_outs),
            out_specs=(PartitionSpec("core"),) * n_outs,
            check_rep=False,
        ),
        donate_argnums=tuple(range(n_params, n_params + n_outs)),
        keep_unused=True,
    )
    sh = NamedSharding(mesh, PartitionSpec("core"))
    dev_in = [
        jax.device_put(
            np.concatenate([np.asarray(m[name]) for m in in_maps], axis=0), sh
        )
        for name in in_names
    ]

    def zeros():
        return [
            jax.device_put(
                np.zeros((N_CORES * z.shape[0], *z.shape[1:]), z.dtype), sh
            )
            for z in zero_outs
        ]

    # warmup (compiles)
    out = sharded(*dev_in, *zeros())
    jax.block_until_ready(out)

    times = []
    for _ in range(n_iters):
        zs = zeros()
        jax.block_until_ready(zs)
        t0 = time.perf_counter()
        out = sharded(*dev_in, *zs)
        jax.block_until_ready(out)
        times.append(time.perf_counter() - t0)
    times.sort()
    return times[0], times[len(times) // 2]
